# revision 1
# baseline (speedup 1.0000x reference)
"""Trainium2 Bass kernel for nn_CerberusSemanticIDBranch (vq_codebook).

Reference semantics (per group g with prototypes P_g [K_g, D]):
    xn = x / (||x|| + 1e-6)
    logits = (xn @ l2(P_g).T) / tau
    q = softmax(logits)
    q_aff = q @ A_g;  q_aff /= (q_aff.sum(-1) + 1e-6)
    out_g = q_aff @ P_g
stacked over 5 groups -> [B, 5, D].

Key folds (host side):
  * A_g has constant row-sums c_g, so the affinity normalization is a
    constant scale: out_g = q @ W_g with W_g = (A_g/(c_g+1e-6)) @ P_g.
  * l2(P) is precomputed; 1/tau and 1/||x|| fold into one per-row scale
    s_b = 1/(tau*||x_b||), applied to the logits.
  * softmax denominator via exp/ln only (single ACT table set):
    s = exp(-0.5*ln(tau^2*ssq)), inv = exp(-ln(segsum)).

Device layout: x is pre-transposed on the host so D sits on SBUF
partitions; batch stays on the free dim through the logits/softmax,
so per-group segment sums are partition-axis sums done as PE matmuls
with 0/1 indicator matrices.  The 5 groups are padded into two
128-partition layouts (A: top@0, pants@64 / B: gender@0, hair@32,
shoes@64) so every matmul operand has a 0/32/64-aligned base partition.

Data parallel over 8 NeuronCores: core i handles rows [i*4096, (i+1)*4096).
"""

import itertools
import sys

import numpy as np

sys.path.insert(0, "/opt/trn_rl_repo")

import concourse.bass as bass  # noqa: E402
import concourse.tile as tile  # noqa: E402
from concourse import mybir  # noqa: E402
from concourse.vector_clock import ScopedClock  # noqa: E402

# ---------------------------------------------------------------- problem
GROUP_DIMS = {
    "gender": [2],
    "hair": [5, 3],
    "top": [8, 5],
    "pants": [8, 5],
    "shoes": [6, 4],
}
TAU = 0.07
B, D = 32768, 512
N_CORES = 8
B_CORE = B // N_CORES          # 4096
SUPER = 512                    # batch rows per supertile
N_SUPER = B_CORE // SUPER      # 8
N_CHUNK = D // 128             # 4
N_GROUPS = 5

# group -> (K, layout, base partition); layout A=0, B=1
GROUP_PLACEMENT = {
    "gender": (2, 1, 0),
    "hair": (15, 1, 32),
    "top": (40, 0, 0),
    "pants": (40, 0, 64),
    "shoes": (24, 1, 64),
}

# const blob column offsets (fp32 columns of a [128, CONST_COLS] tensor)
_PNT_OFF = 0                       # 4 chunks x (A 128 | B 128) = 1024
_W_OFF = 1024                      # 2 layouts x 512
_IND_OFF = 2048                    # 2 layouts x 5
_INDT_OFF = 2058                   # 2 layouts x 128 (rows 0:5)
_ONES_OFF = 2314                   # [128, 128] of ones
CONST_COLS = 2442
# f32r const blob: pnt/W regions + ones column + ind/indt copies
_R_ONES_OFF = 2048
_R_IND_OFF = 2049
_R_INDT_OFF = 2059
CONSTR_COLS = 2315

# dtype knobs for the PE matmuls: "f32" (exact, 4 cyc/row) or
# "f32r" (1 cyc/row at N>=256, ~1e-3 elementwise multiply precision)
MM_DTYPES = {
    "ssq": "f32r",
    "mm1": "f32r",
    "seg": "f32",
    "bcast": "f32",    # S broadcast must stay exact (multiplies logits)
    "invb": "f32",
    "mm2": "f32r",
}

PSUM_BUFS = {"small": 2, "raw": 1, "bcast": 1, "out": 2}  # out slots are 2 banks

_F32 = mybir.dt.float32
_F32R = mybir.dt.float32r
_EXP = mybir.ActivationFunctionType.Exp
_LN = mybir.ActivationFunctionType.Ln


def _is_r(kind):
    return MM_DTYPES[kind] == "f32r"


# ------------------------------------------------------------- tile patch
_NOP_ID = [0]


def _spread_all_waits(nc, max_waits=1):
    """This walrus build rejects instructions carrying more than one sync
    wait (setupSyncWait: "Too many sync wait commands").  Rewrite every
    block so extra waits ride on dedicated same-engine NOPs placed just
    before the instruction (engine queues are FIFO, so semantics hold)."""
    for fn in nc.m.functions:
        for blk in fn.blocks:
            insts = list(blk.instructions)
            out = []
            changed = False
            for inst in insts:
                si = inst.sync_info
                waits = list(si.on_wait) if si is not None and si.on_wait else []
                if len(waits) > max_waits:
                    changed = True
                    for w in waits[:-max_waits]:
                        _NOP_ID[0] += 1
                        out.append(
                            mybir.InstNoOp(
                                name=f"waitnop-{_NOP_ID[0]}",
                                engine=inst.engine,
                                bass_nofuse=True,
                                sync_info=mybir.SyncInfo(
                                    on_wait=[w], on_update=[]),
                            ))
                    si.on_wait = waits[-max_waits:]
                out.append(inst)
            if changed:
                blk.instructions = out


def _patched_drain_and_barrier(self, tick_clock, wait_clock):
    probe = self.nc.sync.nop(nofuse=True)
    wait_clock.add_sem_waits(probe.ins, ScopedClock({None: tick_clock.global_clock}))
    drain_inst = self.nc.sync.drain()
    del drain_inst
    self.nc.all_engine_barrier()
    assert self.sems is not None
    popped = self.nc._tile_sem_poison_stack.pop()
    assert popped is self._sem_poison
    self.nc.clear_and_free_semaphores(list(self.sems.allocated().values()))
    self.nc.all_engine_barrier()
    _spread_all_waits(self.nc)


_patched = False


def _install_tile_patch():
    global _patched
    if not _patched:
        tile.TileContext._drain_and_barrier = _patched_drain_and_barrier
        _patched = True


# --------------------------------------------------------- host constants
def _affinity(dims):
    combos = np.array(
        list(itertools.product(*[range(d) for d in dims])), dtype=np.int32
    )
    return (combos[:, None, :] == combos[None, :, :]).mean(-1).astype(np.float64)


def build_host_constants(protos):
    """protos: dict name -> P_g [K_g, D] float32. Returns the [128, CONST_COLS]
    fp32 constant blob shared by all cores."""
    blob = np.zeros((128, CONST_COLS), dtype=np.float32)

    pn_pad = np.zeros((2, 128, D), dtype=np.float32)   # l2-normalized, padded
    w_pad = np.zeros((2, 128, D), dtype=np.float32)    # (A/c) @ P, padded
    ind = np.zeros((2, 128, N_GROUPS), dtype=np.float32)
    indt = np.zeros((2, N_GROUPS, 128), dtype=np.float32)

    for g, name in enumerate(GROUP_DIMS):
        P = np.asarray(protos[name], dtype=np.float32)
        K, layout, base = GROUP_PLACEMENT[name]
        assert P.shape == (K, D)
        norm = np.linalg.norm(P, axis=-1, keepdims=True).astype(np.float32)
        pn = P / (norm + np.float32(1e-6))
        A = _affinity(GROUP_DIMS[name])                 # [K, K] float64
        c = A[0].sum() + 1e-6                            # constant row sum
        W = ((A / c) @ P.astype(np.float64)).astype(np.float32)
        pn_pad[layout, base : base + K] = pn
        w_pad[layout, base : base + K] = W
        ind[layout, base : base + K, g] = 1.0
        indt[layout, g, base : base + K] = 1.0

    for c in range(N_CHUNK):
        for l in range(2):
            # pnt chunk: [128 d_local, 128 padded-row] = pn_pad[l][:, chunk].T
            blob[:, _PNT_OFF + c * 256 + l * 128 : _PNT_OFF + c * 256 + (l + 1) * 128] = (
                pn_pad[l][:, c * 128 : (c + 1) * 128].T
            )
    for l in range(2):
        blob[:, _W_OFF + l * D : _W_OFF + (l + 1) * D] = w_pad[l]
        blob[:, _IND_OFF + l * N_GROUPS : _IND_OFF + (l + 1) * N_GROUPS] = ind[l]
        blob[0:N_GROUPS, _INDT_OFF + l * 128 : _INDT_OFF + (l + 1) * 128] = indt[l]
    blob[:, _ONES_OFF : _ONES_OFF + 128] = 1.0
    # f32r twin: pnt + W + ones + indicators (consumed by f32r matmuls)
    blobr = np.zeros((128, CONSTR_COLS), dtype=np.float32)
    blobr[:, 0:2048] = blob[:, 0:2048]
    blobr[:, _R_ONES_OFF] = 1.0
    blobr[:, _R_IND_OFF : _R_IND_OFF + 2 * N_GROUPS] = blob[
        :, _IND_OFF : _IND_OFF + 2 * N_GROUPS]
    blobr[:, _R_INDT_OFF : _R_INDT_OFF + 256] = blob[
        :, _INDT_OFF : _INDT_OFF + 256]
    return blob, blobr


# ------------------------------------------------------------ bass program
def build_program(loop_k=None, ablate=None, repeat=1):
    """Emit the SPMD program. loop_k: if set, wrap the whole body in a
    tc.For_i repeat for delta-timing (adds a per-iteration all-engine
    barrier). repeat: python-unrolled repeats (barrier-free, matches the
    real kernel's pipelining). ablate: None | "dma_only" | "no_outdma"
    (perf diagnostics; wrong results)."""
    _install_tile_patch()
    nc = bass.Bass("TRN2", target_bir_lowering=False, debug=False,
                   num_devices=N_CORES)
    # xt is declared f32r so it can directly feed f32r matmuls; bitwise
    # identical to f32 (f32-consuming ops read it via bitcast).
    xt_d = nc.dram_tensor("xt", [D, B_CORE], _F32R, kind="ExternalInput").ap()
    const_d = nc.dram_tensor("consts", [128, CONST_COLS], _F32,
                             kind="ExternalInput").ap()
    constr_d = nc.dram_tensor("constsr", [128, CONSTR_COLS], _F32R,
                              kind="ExternalInput").ap()
    out_d = nc.dram_tensor("out", [B_CORE, N_GROUPS, D], _F32,
                           kind="ExternalOutput").ap()

    with tile.TileContext(nc) as tc:
        import contextlib

        with contextlib.ExitStack() as ctx:
            cpool = ctx.enter_context(tc.tile_pool(name="consts", bufs=1))
            xt_pool = ctx.enter_context(tc.tile_pool(name="xt", bufs=4))
            sq_pool = ctx.enter_context(tc.tile_pool(name="sq", bufs=2))
            work = ctx.enter_context(tc.tile_pool(name="work", bufs=3))
            et_pool = ctx.enter_context(tc.tile_pool(name="et", bufs=4))
            qt_pool = ctx.enter_context(tc.tile_pool(name="qt", bufs=4))
            tiny = ctx.enter_context(tc.tile_pool(name="tiny", bufs=6))
            stage = ctx.enter_context(tc.tile_pool(name="stage", bufs=8))
            ps_small = ctx.enter_context(
                tc.tile_pool(name="ps_small", bufs=PSUM_BUFS["small"],
                             space="PSUM"))
            ps_raw = ctx.enter_context(
                tc.tile_pool(name="ps_raw", bufs=PSUM_BUFS["raw"],
                             space="PSUM"))
            ps_bcast = ctx.enter_context(
                tc.tile_pool(name="ps_bcast", bufs=PSUM_BUFS["bcast"],
                             space="PSUM"))
            ps_out = ctx.enter_context(
                tc.tile_pool(name="ps_out", bufs=PSUM_BUFS["out"],
                             space="PSUM"))

            consts = cpool.tile([128, CONST_COLS], _F32)
            nc.sync.dma_start(out=consts[:], in_=const_d[:])
            constsr = cpool.tile([128, CONSTR_COLS], _F32R)
            nc.sync.dma_start(out=constsr[:], in_=constr_d[:])

            def pnt(c, l):
                o = _PNT_OFF + c * 256 + l * 128
                src = constsr if _is_r("mm1") else consts
                return src[:, o : o + 128]

            def w_l(l, base, K):
                src = constsr if _is_r("mm2") else consts
                return src[base : base + K, _W_OFF + l * D : _W_OFF + (l + 1) * D]

            def ind_l(l):
                if _is_r("seg"):
                    o = _R_IND_OFF + l * N_GROUPS
                    return constsr[:, o : o + N_GROUPS]
                o = _IND_OFF + l * N_GROUPS
                return consts[:, o : o + N_GROUPS]

            def indt_l(l):
                if _is_r("invb"):
                    o = _R_INDT_OFF + l * 128
                    return constsr[0:N_GROUPS, o : o + 128]
                o = _INDT_OFF + l * 128
                return consts[0:N_GROUPS, o : o + 128]

            ones_col = (constsr[:, _R_ONES_OFF : _R_ONES_OFF + 1]
                        if _is_r("ssq")
                        else consts[:, _ONES_OFF : _ONES_OFF + 1])  # [128,1]
            ones_row = consts[0:1, _ONES_OFF : _ONES_OFF + 128]     # [1,128]

            def supertile(s):
                b0 = s * SUPER
                xt = xt_pool.tile([128, N_CHUNK, SUPER], _F32R, tag="xt")
                nc.sync.dma_start(
                    out=xt[:],
                    in_=xt_d[:, b0 : b0 + SUPER].rearrange(
                        "(c p) b -> p c b", c=N_CHUNK),
                )
                if ablate == "dma_only":
                    for j in range(SUPER // 128):
                        st = stage.tile([128, N_GROUPS, D], _F32, tag="stage")
                        nc.vector.tensor_copy(st[:, 0, 0:4], xt[0:128, 0, 0:4])
                        nc.sync.dma_start(
                            out=out_d[b0 + j * 128 : b0 + (j + 1) * 128],
                            in_=st[:])
                    return
                xt32 = xt[:].bitcast(_F32)

                def xt_mm(c, kind):
                    return xt[:, c] if _is_r(kind) else xt32[:, c]

                # squares (gpsimd, SBUF only) + ssq = ones^T . sq  (PE)
                # per-chunk ops so each ssq matmul can start early
                sq = sq_pool.tile([128, N_CHUNK, SUPER],
                                  _F32R if _is_r("ssq") else _F32, tag="sq")
                for c in range(N_CHUNK):
                    nc.gpsimd.tensor_mul(sq[:, c], xt32[:, c], xt32[:, c])
                ssq = ps_small.tile([1, SUPER], _F32, tag="small")
                for c in range(N_CHUNK):
                    nc.tensor.matmul(
                        ssq[:], ones_col, sq[:, c],
                        start=(c == 0), stop=(c == N_CHUNK - 1))
                # s = exp(-0.5 * ln(tau^2 * ssq)) = 1/(tau*||x||)
                t1 = tiny.tile([1, SUPER], _F32, tag="tiny")
                nc.scalar.activation(t1[:], ssq[:], _LN, scale=float(TAU * TAU))
                s_t = tiny.tile([1, SUPER], _F32, tag="tiny")
                nc.scalar.activation(s_t[:], t1[:], _EXP, scale=-0.5)
                # S = broadcast of s to 128 partitions (PE, exact in f32)
                S_ps = ps_small.tile([128, SUPER], _F32, tag="small")
                nc.tensor.matmul(S_ps[:], ones_row, s_t[:], start=True, stop=True)
                S_sb = work.tile([128, SUPER], _F32, tag="S")
                nc.scalar.copy(S_sb[:], S_ps[:])

                # logits^T per layout, exp
                ets = []
                for l in range(2):
                    raw = ps_raw.tile([128, SUPER], _F32, tag="raw")
                    for c in range(N_CHUNK):
                        nc.tensor.matmul(
                            raw[:], pnt(c, l), xt_mm(c, "mm1"),
                            start=(c == 0), stop=(c == N_CHUNK - 1))
                    rawS = work.tile([128, SUPER], _F32, tag="rawS")
                    nc.vector.tensor_mul(rawS[:], raw[:], S_sb[:])
                    et = et_pool.tile([128, SUPER],
                                      _F32R if _is_r("seg") else _F32,
                                      tag="et")
                    nc.scalar.activation(et[:], rawS[:], _EXP)
                    ets.append(et)

                # segment sums over both layouts -> [5, SUPER]
                sums = ps_small.tile([N_GROUPS, SUPER], _F32, tag="small")
                for l in range(2):
                    nc.tensor.matmul(sums[:], ind_l(l), ets[l][:],
                                     start=(l == 0), stop=(l == 1))
                # inv = exp(-ln(sums)) = 1/sums
                t2 = tiny.tile([N_GROUPS, SUPER], _F32, tag="tiny")
                nc.scalar.activation(t2[:], sums[:], _LN)
                inv = tiny.tile([N_GROUPS, SUPER],
                                _F32R if _is_r("invb") else _F32, tag="tiny")
                nc.scalar.activation(inv[:], t2[:], _EXP, scale=-1.0)

                # qT per layout = eT * broadcast(inv by group rows)
                qt_dt = _F32R if _is_r("mm2") else _F32
                qts = []
                for l in range(2):
                    invB = ps_bcast.tile([128, SUPER], _F32, tag="invB")
                    nc.tensor.matmul(invB[:], indt_l(l), inv[:],
                                     start=True, stop=True)
                    qt = qt_pool.tile([128, SUPER], qt_dt, tag="qt")
                    nc.vector.tensor_mul(qt[:], ets[l][:], invB[:])
                    qts.append(qt)

                # second matmul: out[b, g, :] = q_g @ W_g, per 128-row tile.
                # Adjacent groups share a 2-bank PSUM tile so evacuation is
                # one [128,1024] copy per pair (fewer DVE drains/instrs).
                groups = list(GROUP_PLACEMENT.items())
                for j in range(SUPER // 128):
                    st = stage.tile([128, N_GROUPS, D], _F32, tag="stage")

                    def mm2(ps_slice, gi):
                        name, (K, l, base) = groups[gi]
                        nc.tensor.matmul(
                            ps_slice,
                            qts[l][base : base + K, j * 128 : (j + 1) * 128],
                            w_l(l, base, K),
                            start=True, stop=True)

                    pr0 = ps_out.tile([128, 2, D], _F32, tag="out")
                    mm2(pr0[:, 0, :], 0)
                    mm2(pr0[:, 1, :], 1)
                    nc.vector.tensor_copy(st[:, 0:2, :], pr0[:])
                    pr1 = ps_out.tile([128, 2, D], _F32, tag="out")
                    mm2(pr1[:, 0, :], 2)
                    mm2(pr1[:, 1, :], 3)
                    nc.scalar.copy(st[:, 2:4, :], pr1[:])
                    pr2 = ps_out.tile([128, 2, D], _F32, tag="out")
                    mm2(pr2[:, 0, :], 4)
                    nc.vector.tensor_copy(st[:, 4, :], pr2[:, 0, :])
                    if ablate != "no_outdma":
                        nc.sync.dma_start(
                            out=out_d[b0 + j * 128 : b0 + (j + 1) * 128],
                            in_=st[:])

            if loop_k is None:
                for _ in range(repeat):
                    for s in range(N_SUPER):
                        supertile(s)
            else:
                with tc.For_i(0, loop_k, 1):
                    for s in range(N_SUPER):
                        supertile(s)

    return nc


# ------------------------------------------------------------- entry point
def make_core_inputs(inputs):
    """Full inputs dict -> list of 8 per-core input maps."""
    x = np.asarray(inputs["x"], dtype=np.float32)
    assert x.shape == (B, D)
    protos = {k: inputs[f"P_{k}"] for k in GROUP_DIMS}
    blob, blobr = build_host_constants(protos)
    maps = []
    for i in range(N_CORES):
        shard = x[i * B_CORE : (i + 1) * B_CORE]
        maps.append({"xt": np.ascontiguousarray(shard.T), "consts": blob,
                     "constsr": blobr})
    return maps


_CACHE = {}


def kernel(x, P_gender, P_hair, P_top, P_pants, P_shoes):
    from concourse.bass_utils import run_bass_kernel_spmd

    inputs = dict(x=x, P_gender=P_gender, P_hair=P_hair, P_top=P_top,
                  P_pants=P_pants, P_shoes=P_shoes)
    if "nc" not in _CACHE:
        _CACHE["nc"] = build_program()
    in_maps = make_core_inputs(inputs)
    res = run_bass_kernel_spmd(_CACHE["nc"], in_maps, list(range(N_CORES)))
    return np.concatenate([res.results[i]["out"] for i in range(N_CORES)], axis=0)



# revision 7
# speedup vs baseline: 3.7870x; 3.7870x over previous
"""Trainium2 Bass kernel for nn_CerberusSemanticIDBranch (vq_codebook).

Reference semantics (per group g with prototypes P_g [K_g, D]):
    xn = x / (||x|| + 1e-6)
    logits = (xn @ l2(P_g).T) / tau
    q = softmax(logits)
    q_aff = q @ A_g;  q_aff /= (q_aff.sum(-1) + 1e-6)
    out_g = q_aff @ P_g
stacked over 5 groups -> [B, 5, D].

Key folds (host side):
  * A_g has constant row-sums c_g, so the affinity normalization is a
    constant scale: out_g = q @ W_g with W_g = (A_g/(c_g+1e-6)) @ P_g.
  * l2(P) is precomputed; 1/tau and 1/||x|| fold into one per-row scale
    s_b = 1/(tau*||x_b||), applied to the logits.
  * softmax denominator via exp/ln only (single ACT table set):
    s = exp(-0.5*ln(tau^2*ssq)), inv = exp(-ln(segsum)).

Memory regime: the kernel is HBM-bound (out is 41.9MB/core in fp32), so
x, the prototype constants, and the output all travel as bf16 (the
harness gate is max|err|/absmax < 2e-2; bf16 costs ~3e-3).  The output
is written in a partition-contiguous layout ([N_SUPER, 128, 4*5*512],
one 2.6MB DMA per supertile with 20KB/partition descriptors) and
unshuffled + upcast on the host.

Device layout: x is pre-transposed on the host so D sits on SBUF
partitions; batch stays on the free dim through the logits/softmax,
so per-group segment sums are partition-axis sums done as PE matmuls
with 0/1 indicator matrices.  The 5 groups are padded into two
128-partition layouts (A: top@0, pants@64 / B: gender@0, hair@32,
shoes@64) so every matmul operand has a 0/32/64-aligned base partition.

Data parallel over 8 NeuronCores: core i handles rows [i*4096, (i+1)*4096).
"""

import itertools
import sys

import numpy as np

sys.path.insert(0, "/opt/trn_rl_repo")

import concourse.bass as bass  # noqa: E402
import concourse.tile as tile  # noqa: E402
from concourse import mybir  # noqa: E402
from concourse.vector_clock import ScopedClock  # noqa: E402

# ---------------------------------------------------------------- problem
GROUP_DIMS = {
    "gender": [2],
    "hair": [5, 3],
    "top": [8, 5],
    "pants": [8, 5],
    "shoes": [6, 4],
}
TAU = 0.07
B, D = 32768, 512
N_CORES = 8
B_CORE = B // N_CORES          # 4096
SUPER = 512                    # batch rows per supertile
N_SUPER = B_CORE // SUPER      # 8
N_CHUNK = D // 128             # 4
N_GROUPS = 5
N_J = SUPER // 128             # 4 row-tiles per supertile

# group -> (K, layout, base partition); layout A=0, B=1
GROUP_PLACEMENT = {
    "gender": (2, 1, 0),
    "hair": (15, 1, 32),
    "top": (40, 0, 0),
    "pants": (40, 0, 64),
    "shoes": (24, 1, 64),
}

# bf16 const blob column offsets ([128, BF_COLS] bfloat16)
_PNT_OFF = 0                       # 4 chunks x (A 128 | B 128) = 1024
_W_OFF = 1024                      # 2 layouts x 512
BF_COLS = 2048
# f32 const blob column offsets ([128, F32_COLS] float32)
_IND_OFF = 0                       # 2 layouts x 5
_INDT_OFF = 10                     # 2 layouts x 128 (rows 0:5)
_ONES_OFF = 266                    # [128, 128] of ones
F32_COLS = 394

PSUM_BUFS = {"small": 2, "raw": 1, "bcast": 1, "out": 2}  # out slots are 2 banks

_F32 = mybir.dt.float32
_F32R = mybir.dt.float32r
_BF16 = mybir.dt.bfloat16
_EXP = mybir.ActivationFunctionType.Exp
_LN = mybir.ActivationFunctionType.Ln


# ------------------------------------------------------------- tile patch
_NOP_ID = [0]


def _spread_all_waits(nc, max_waits=1):
    """This walrus build rejects instructions carrying more than one sync
    wait (setupSyncWait: "Too many sync wait commands").  Rewrite every
    block so extra waits ride on dedicated same-engine NOPs placed just
    before the instruction (engine queues are FIFO, so semantics hold)."""
    for fn in nc.m.functions:
        for blk in fn.blocks:
            insts = list(blk.instructions)
            out = []
            changed = False
            for inst in insts:
                si = inst.sync_info
                waits = list(si.on_wait) if si is not None and si.on_wait else []
                if len(waits) > max_waits:
                    changed = True
                    for w in waits[:-max_waits]:
                        _NOP_ID[0] += 1
                        out.append(
                            mybir.InstNoOp(
                                name=f"waitnop-{_NOP_ID[0]}",
                                engine=inst.engine,
                                bass_nofuse=True,
                                sync_info=mybir.SyncInfo(
                                    on_wait=[w], on_update=[]),
                            ))
                    si.on_wait = waits[-max_waits:]
                out.append(inst)
            if changed:
                blk.instructions = out


def _patched_drain_and_barrier(self, tick_clock, wait_clock):
    probe = self.nc.sync.nop(nofuse=True)
    wait_clock.add_sem_waits(probe.ins, ScopedClock({None: tick_clock.global_clock}))
    drain_inst = self.nc.sync.drain()
    del drain_inst
    self.nc.all_engine_barrier()
    assert self.sems is not None
    popped = self.nc._tile_sem_poison_stack.pop()
    assert popped is self._sem_poison
    self.nc.clear_and_free_semaphores(list(self.sems.allocated().values()))
    self.nc.all_engine_barrier()
    _spread_all_waits(self.nc)


_patched = False


def _install_tile_patch():
    global _patched
    if not _patched:
        tile.TileContext._drain_and_barrier = _patched_drain_and_barrier
        _patched = True


# --------------------------------------------------------- host constants
def _affinity(dims):
    combos = np.array(
        list(itertools.product(*[range(d) for d in dims])), dtype=np.int32
    )
    return (combos[:, None, :] == combos[None, :, :]).mean(-1).astype(np.float64)


def _bf16(a):
    import ml_dtypes

    return np.asarray(a, dtype=np.float32).astype(ml_dtypes.bfloat16)


def build_host_constants(protos):
    """protos: dict name -> P_g [K_g, D] float32. Returns the bf16
    [128, BF_COLS] and f32 [128, F32_COLS] constant blobs shared by all
    cores."""
    import ml_dtypes

    blob32 = np.zeros((128, F32_COLS), dtype=np.float32)
    blob16 = np.zeros((128, BF_COLS), dtype=ml_dtypes.bfloat16)

    pn_pad = np.zeros((2, 128, D), dtype=np.float32)   # l2-normalized, padded
    w_pad = np.zeros((2, 128, D), dtype=np.float32)    # (A/c) @ P, padded
    ind = np.zeros((2, 128, N_GROUPS), dtype=np.float32)
    indt = np.zeros((2, N_GROUPS, 128), dtype=np.float32)

    for g, name in enumerate(GROUP_DIMS):
        P = np.asarray(protos[name], dtype=np.float32)
        K, layout, base = GROUP_PLACEMENT[name]
        assert P.shape == (K, D)
        norm = np.linalg.norm(P, axis=-1, keepdims=True).astype(np.float32)
        pn = P / (norm + np.float32(1e-6))
        A = _affinity(GROUP_DIMS[name])                 # [K, K] float64
        c = A[0].sum() + 1e-6                            # constant row sum
        W = ((A / c) @ P.astype(np.float64)).astype(np.float32)
        pn_pad[layout, base : base + K] = pn
        w_pad[layout, base : base + K] = W
        ind[layout, base : base + K, g] = 1.0
        indt[layout, g, base : base + K] = 1.0

    for c in range(N_CHUNK):
        for l in range(2):
            # pnt chunk: [128 d_local, 128 padded-row] = pn_pad[l][:, chunk].T
            blob16[:, _PNT_OFF + c * 256 + l * 128 : _PNT_OFF + c * 256 + (l + 1) * 128] = (
                _bf16(pn_pad[l][:, c * 128 : (c + 1) * 128].T)
            )
    for l in range(2):
        blob16[:, _W_OFF + l * D : _W_OFF + (l + 1) * D] = _bf16(w_pad[l])
        blob32[:, _IND_OFF + l * N_GROUPS : _IND_OFF + (l + 1) * N_GROUPS] = ind[l]
        blob32[0:N_GROUPS, _INDT_OFF + l * 128 : _INDT_OFF + (l + 1) * 128] = indt[l]
    blob32[:, _ONES_OFF : _ONES_OFF + 128] = 1.0
    return blob32, blob16


# ------------------------------------------------------------ bass program
def build_program(loop_k=None, ablate=None, repeat=1):
    """Emit the SPMD program. loop_k: if set, wrap the whole body in a
    tc.For_i repeat for delta-timing (adds a per-iteration all-engine
    barrier). repeat: python-unrolled repeats (barrier-free, matches the
    real kernel's pipelining). ablate: None | "dma_only" | "no_outdma"
    (perf diagnostics; wrong results)."""
    _install_tile_patch()
    nc = bass.Bass("TRN2", target_bir_lowering=False, debug=False,
                   num_devices=N_CORES)
    # x, pre-transposed + bf16 on host: partition p of pair r holds the
    # two supertiles' chunks contiguously (2*4*512 bf16 = 8KB descriptors)
    xt_d = nc.dram_tensor("xt", [N_SUPER // 2, 128, 2 * N_CHUNK * SUPER],
                          _BF16, kind="ExternalInput").ap()
    const32_d = nc.dram_tensor("consts32", [128, F32_COLS], _F32,
                               kind="ExternalInput").ap()
    const16_d = nc.dram_tensor("consts16", [128, BF_COLS], _BF16,
                               kind="ExternalInput").ap()
    onesr_d = nc.dram_tensor("onesr", [128, 1], _F32R,
                             kind="ExternalInput").ap()
    # out, partition-contiguous per supertile: row (s, p) holds the 4
    # j-tiles' [5, 512] slabs for batch rows s*512 + j*128 + p (20KB descs)
    out_d = nc.dram_tensor("out", [N_SUPER, 128, N_J * N_GROUPS * D],
                           _BF16, kind="ExternalOutput").ap()

    with tile.TileContext(nc) as tc:
        import contextlib

        with contextlib.ExitStack() as ctx:
            cpool = ctx.enter_context(tc.tile_pool(name="consts", bufs=1))
            xt_pool = ctx.enter_context(tc.tile_pool(name="xt", bufs=2))
            sq_pool = ctx.enter_context(tc.tile_pool(name="sq", bufs=2))
            work = ctx.enter_context(tc.tile_pool(name="work", bufs=3))
            et_pool = ctx.enter_context(tc.tile_pool(name="et", bufs=4))
            qt_pool = ctx.enter_context(tc.tile_pool(name="qt", bufs=4))
            tiny = ctx.enter_context(tc.tile_pool(name="tiny", bufs=6))
            stage = ctx.enter_context(tc.tile_pool(name="stage", bufs=2))
            ps_small = ctx.enter_context(
                tc.tile_pool(name="ps_small", bufs=PSUM_BUFS["small"],
                             space="PSUM"))
            ps_raw = ctx.enter_context(
                tc.tile_pool(name="ps_raw", bufs=PSUM_BUFS["raw"],
                             space="PSUM"))
            ps_bcast = ctx.enter_context(
                tc.tile_pool(name="ps_bcast", bufs=PSUM_BUFS["bcast"],
                             space="PSUM"))
            ps_out = ctx.enter_context(
                tc.tile_pool(name="ps_out", bufs=PSUM_BUFS["out"],
                             space="PSUM"))

            consts32 = cpool.tile([128, F32_COLS], _F32)
            nc.sync.dma_start(out=consts32[:], in_=const32_d[:])
            consts16 = cpool.tile([128, BF_COLS], _BF16)
            nc.sync.dma_start(out=consts16[:], in_=const16_d[:])

            def pnt(c, l):
                o = _PNT_OFF + c * 256 + l * 128
                return consts16[:, o : o + 128]

            def w_l(l, base, K):
                return consts16[base : base + K,
                                _W_OFF + l * D : _W_OFF + (l + 1) * D]

            def ind_l(l):
                o = _IND_OFF + l * N_GROUPS
                return consts32[:, o : o + N_GROUPS]

            def indt_l(l):
                o = _INDT_OFF + l * 128
                return consts32[0:N_GROUPS, o : o + 128]

            onesr = cpool.tile([128, 1], _F32R)
            nc.sync.dma_start(out=onesr[:], in_=onesr_d[:])
            ones_col_r = onesr[:]
            ones_row = consts32[0:1, _ONES_OFF : _ONES_OFF + 128]   # [1,128]

            def supertile(s, xt2):
                b0 = s * SUPER
                xt = xt2[:, s % 2]          # [128, N_CHUNK, SUPER] bf16
                if ablate == "dma_only":
                    st = stage.tile([128, N_J, N_GROUPS, D], _BF16,
                                    tag="stage")
                    nc.vector.tensor_copy(st[:, 0, 0, 0:4], xt[0:128, 0, 0:4])
                    nc.sync.dma_start(out=out_d[s], in_=st[:].rearrange(
                        "p j g d -> p (j g d)"))
                    return

                # squares (gpsimd, SBUF only) + ssq = ones^T . sq  (PE)
                # per-chunk ops so each ssq matmul can start early
                sq = sq_pool.tile([128, N_CHUNK, SUPER], _F32R, tag="sq")
                for c in range(N_CHUNK):
                    nc.gpsimd.tensor_mul(sq[:, c], xt[:, c], xt[:, c])
                ssq = ps_small.tile([1, SUPER], _F32, tag="small")
                for c in range(N_CHUNK):
                    nc.tensor.matmul(
                        ssq[:], ones_col_r, sq[:, c],
                        start=(c == 0), stop=(c == N_CHUNK - 1))
                # s = exp(-0.5 * ln(tau^2 * ssq)) = 1/(tau*||x||)
                t1 = tiny.tile([1, SUPER], _F32, tag="tiny")
                nc.scalar.activation(t1[:], ssq[:], _LN, scale=float(TAU * TAU))
                s_t = tiny.tile([1, SUPER], _F32, tag="tiny")
                nc.scalar.activation(s_t[:], t1[:], _EXP, scale=-0.5)
                # S = broadcast of s to 128 partitions (PE, exact in f32)
                S_ps = ps_small.tile([128, SUPER], _F32, tag="small")
                nc.tensor.matmul(S_ps[:], ones_row, s_t[:], start=True, stop=True)
                S_sb = work.tile([128, SUPER], _F32, tag="S")
                nc.scalar.copy(S_sb[:], S_ps[:])

                # logits^T per layout (bf16 matmul), exp
                ets = []
                for l in range(2):
                    raw = ps_raw.tile([128, SUPER], _F32, tag="raw")
                    for c in range(N_CHUNK):
                        nc.tensor.matmul(
                            raw[:], pnt(c, l), xt[:, c],
                            start=(c == 0), stop=(c == N_CHUNK - 1))
                    rawS = work.tile([128, SUPER], _F32, tag="rawS")
                    nc.vector.tensor_mul(rawS[:], raw[:], S_sb[:])
                    et = et_pool.tile([128, SUPER], _F32, tag="et")
                    nc.scalar.activation(et[:], rawS[:], _EXP)
                    ets.append(et)

                # segment sums over both layouts -> [5, SUPER]
                sums = ps_small.tile([N_GROUPS, SUPER], _F32, tag="small")
                for l in range(2):
                    nc.tensor.matmul(sums[:], ind_l(l), ets[l][:],
                                     start=(l == 0), stop=(l == 1))
                # inv = exp(-ln(sums)) = 1/sums
                t2 = tiny.tile([N_GROUPS, SUPER], _F32, tag="tiny")
                nc.scalar.activation(t2[:], sums[:], _LN)
                inv = tiny.tile([N_GROUPS, SUPER], _F32, tag="tiny")
                nc.scalar.activation(inv[:], t2[:], _EXP, scale=-1.0)

                # qT per layout = eT * broadcast(inv by group rows), bf16
                qts = []
                for l in range(2):
                    invB = ps_bcast.tile([128, SUPER], _F32, tag="invB")
                    nc.tensor.matmul(invB[:], indt_l(l), inv[:],
                                     start=True, stop=True)
                    qt = qt_pool.tile([128, SUPER], _BF16, tag="qt")
                    nc.vector.tensor_mul(qt[:], ets[l][:], invB[:])
                    qts.append(qt)

                # second matmul: out[b, g, :] = q_g @ W_g, per 128-row tile.
                # Adjacent groups share a 2-bank PSUM tile so evacuation is
                # one [128,1024] copy per pair (fewer DVE drains/instrs).
                # All 4 j-tiles land in one [128, 4, 5, 512] bf16 stage
                # buffer -> a single 2.6MB out DMA per supertile.
                groups = list(GROUP_PLACEMENT.items())
                st = stage.tile([128, N_J, N_GROUPS, D], _BF16, tag="stage")
                for j in range(N_J):
                    def mm2(ps_slice, gi):
                        name, (K, l, base) = groups[gi]
                        nc.tensor.matmul(
                            ps_slice,
                            qts[l][base : base + K, j * 128 : (j + 1) * 128],
                            w_l(l, base, K),
                            start=True, stop=True)

                    pr0 = ps_out.tile([128, 2, D], _F32, tag="out")
                    mm2(pr0[:, 0, :], 0)
                    mm2(pr0[:, 1, :], 1)
                    nc.vector.tensor_copy(st[:, j, 0:2, :], pr0[:])
                    pr1 = ps_out.tile([128, 2, D], _F32, tag="out")
                    mm2(pr1[:, 0, :], 2)
                    mm2(pr1[:, 1, :], 3)
                    nc.scalar.copy(st[:, j, 2:4, :], pr1[:])
                    pr2 = ps_out.tile([128, 2, D], _F32, tag="out")
                    mm2(pr2[:, 0, :], 4)
                    nc.vector.tensor_copy(st[:, j, 4, :], pr2[:, 0, :])
                if ablate != "no_outdma":
                    nc.sync.dma_start(
                        out=out_d[s],
                        in_=st[:].rearrange("p j g d -> p (j g d)"))

            def body():
                for r in range(N_SUPER // 2):
                    xt2 = xt_pool.tile([128, 2, N_CHUNK, SUPER], _BF16,
                                       tag="xt")
                    nc.sync.dma_start(
                        out=xt2[:],
                        in_=xt_d[r].rearrange("p (t c b) -> p t c b",
                                              t=2, c=N_CHUNK))
                    supertile(2 * r, xt2)
                    supertile(2 * r + 1, xt2)

            if loop_k is None:
                for _ in range(repeat):
                    body()
            else:
                with tc.For_i(0, loop_k, 1):
                    body()

    return nc


# ------------------------------------------------------------- entry point
def make_core_inputs(inputs):
    """Full inputs dict -> list of 8 per-core input maps."""
    import ml_dtypes

    x = np.asarray(inputs["x"], dtype=np.float32)
    assert x.shape == (B, D)
    protos = {k: inputs[f"P_{k}"] for k in GROUP_DIMS}
    blob32, blob16 = build_host_constants(protos)
    maps = []
    for i in range(N_CORES):
        shard = x[i * B_CORE : (i + 1) * B_CORE]
        # [B_CORE, D] -> xT [D, B_CORE] -> [c, p, r, t*b] -> [r, p, (t c b)]
        xt = shard.T.reshape(N_CHUNK, 128, N_SUPER // 2, 2, SUPER)
        xt = np.ascontiguousarray(xt.transpose(2, 1, 3, 0, 4)).astype(
            ml_dtypes.bfloat16)
        maps.append({
            "xt": xt.reshape(N_SUPER // 2, 128, 2 * N_CHUNK * SUPER),
            "consts32": blob32,
            "consts16": blob16,
            "onesr": np.ones((128, 1), dtype=np.float32),
        })
    return maps


def assemble_output(res_list):
    """Per-core 'out' tensors -> full [B, N_GROUPS, D] float32."""
    outs = []
    for i in range(N_CORES):
        a = np.asarray(res_list[i]["out"]).reshape(
            N_SUPER, 128, N_J, N_GROUPS, D)
        a = a.transpose(0, 2, 1, 3, 4).reshape(B_CORE, N_GROUPS, D)
        outs.append(a.astype(np.float32))
    return np.concatenate(outs, axis=0)


_CACHE = {}


def kernel(x, P_gender, P_hair, P_top, P_pants, P_shoes):
    from concourse.bass_utils import run_bass_kernel_spmd

    inputs = dict(x=x, P_gender=P_gender, P_hair=P_hair, P_top=P_top,
                  P_pants=P_pants, P_shoes=P_shoes)
    if "nc" not in _CACHE:
        _CACHE["nc"] = build_program()
    in_maps = make_core_inputs(inputs)
    res = run_bass_kernel_spmd(_CACHE["nc"], in_maps, list(range(N_CORES)))
    return assemble_output(res.results)


# revision 10
# speedup vs baseline: 3.9123x; 1.0331x over previous
"""Trainium2 Bass kernel for nn_CerberusSemanticIDBranch (vq_codebook).

Reference semantics (per group g with prototypes P_g [K_g, D]):
    xn = x / (||x|| + 1e-6)
    logits = (xn @ l2(P_g).T) / tau
    q = softmax(logits)
    q_aff = q @ A_g;  q_aff /= (q_aff.sum(-1) + 1e-6)
    out_g = q_aff @ P_g
stacked over 5 groups -> [B, 5, D].

Key folds (host side):
  * A_g has constant row-sums c_g, so the affinity normalization is a
    constant scale: out_g = q @ W_g with W_g = (A_g/(c_g+1e-6)) @ P_g.
  * l2(P) is precomputed; 1/tau and 1/||x|| fold into one per-row scale
    s_b = 1/(tau*||x_b||), applied to the logits.
  * softmax denominator via exp/ln only (single ACT table set):
    s = exp(-0.5*ln(tau^2*ssq)), inv = exp(-ln(segsum)).

Memory regime: the kernel is HBM-bound (out is 41.9MB/core in fp32), so
x, the prototype constants, and the output all travel as bf16 (the
harness gate is max|err|/absmax < 2e-2; bf16 costs ~3e-3).  The output
is written in a partition-contiguous layout ([N_SUPER, 128, 4*5*512],
one 2.6MB DMA per supertile with 20KB/partition descriptors) and
unshuffled + upcast on the host.

Device layout: x is pre-transposed on the host so D sits on SBUF
partitions; batch stays on the free dim through the logits/softmax,
so per-group segment sums are partition-axis sums done as PE matmuls
with 0/1 indicator matrices.  The 5 groups are padded into two
128-partition layouts (A: top@0, pants@64 / B: gender@0, hair@32,
shoes@64) so every matmul operand has a 0/32/64-aligned base partition.

Data parallel over 8 NeuronCores: core i handles rows [i*4096, (i+1)*4096).
"""

import itertools
import sys

import numpy as np

sys.path.insert(0, "/opt/trn_rl_repo")

import concourse.bass as bass  # noqa: E402
import concourse.tile as tile  # noqa: E402
from concourse import mybir  # noqa: E402
from concourse.vector_clock import ScopedClock  # noqa: E402

# ---------------------------------------------------------------- problem
GROUP_DIMS = {
    "gender": [2],
    "hair": [5, 3],
    "top": [8, 5],
    "pants": [8, 5],
    "shoes": [6, 4],
}
TAU = 0.07
B, D = 32768, 512
N_CORES = 8
B_CORE = B // N_CORES          # 4096
SUPER = 512                    # batch rows per supertile
N_SUPER = B_CORE // SUPER      # 8
N_CHUNK = D // 128             # 4
N_GROUPS = 5
N_J = SUPER // 128             # 4 row-tiles per supertile

# group -> (K, layout, base partition); layout A=0, B=1
GROUP_PLACEMENT = {
    "gender": (2, 1, 0),
    "hair": (15, 1, 32),
    "top": (40, 0, 0),
    "pants": (40, 0, 64),
    "shoes": (24, 1, 64),
}

# bf16 const blob column offsets ([128, BF_COLS] bfloat16)
_PNT_OFF = 0                       # 4 chunks x (A 128 | B 128) = 1024
_W_OFF = 1024                      # 2 layouts x 512
BF_COLS = 2048
# f32 const blob column offsets ([128, F32_COLS] float32)
_IND_OFF = 0                       # 2 layouts x 5
_INDT_OFF = 10                     # 2 layouts x 128 (rows 0:5)
_ONES_OFF = 266                    # [128, 128] of ones
F32_COLS = 394

PSUM_BUFS = {"small": 2, "raw": 1, "bcast": 1, "out": 2}  # out slots are 2 banks

_F32 = mybir.dt.float32
_F32R = mybir.dt.float32r
_BF16 = mybir.dt.bfloat16
_EXP = mybir.ActivationFunctionType.Exp
_LN = mybir.ActivationFunctionType.Ln


# ------------------------------------------------------------- tile patch
_NOP_ID = [0]


def _spread_all_waits(nc, max_waits=1):
    """This walrus build rejects instructions carrying more than one sync
    wait (setupSyncWait: "Too many sync wait commands").  Rewrite every
    block so extra waits ride on dedicated same-engine NOPs placed just
    before the instruction (engine queues are FIFO, so semantics hold)."""
    for fn in nc.m.functions:
        for blk in fn.blocks:
            insts = list(blk.instructions)
            out = []
            changed = False
            for inst in insts:
                si = inst.sync_info
                waits = list(si.on_wait) if si is not None and si.on_wait else []
                if len(waits) > max_waits:
                    changed = True
                    for w in waits[:-max_waits]:
                        _NOP_ID[0] += 1
                        out.append(
                            mybir.InstNoOp(
                                name=f"waitnop-{_NOP_ID[0]}",
                                engine=inst.engine,
                                bass_nofuse=True,
                                sync_info=mybir.SyncInfo(
                                    on_wait=[w], on_update=[]),
                            ))
                    si.on_wait = waits[-max_waits:]
                out.append(inst)
            if changed:
                blk.instructions = out


def _patched_drain_and_barrier(self, tick_clock, wait_clock):
    probe = self.nc.sync.nop(nofuse=True)
    wait_clock.add_sem_waits(probe.ins, ScopedClock({None: tick_clock.global_clock}))
    drain_inst = self.nc.sync.drain()
    del drain_inst
    self.nc.all_engine_barrier()
    assert self.sems is not None
    popped = self.nc._tile_sem_poison_stack.pop()
    assert popped is self._sem_poison
    self.nc.clear_and_free_semaphores(list(self.sems.allocated().values()))
    self.nc.all_engine_barrier()
    _spread_all_waits(self.nc)


_patched = False


def _install_tile_patch():
    global _patched
    if not _patched:
        tile.TileContext._drain_and_barrier = _patched_drain_and_barrier
        _patched = True


# --------------------------------------------------------- host constants
def _affinity(dims):
    combos = np.array(
        list(itertools.product(*[range(d) for d in dims])), dtype=np.int32
    )
    return (combos[:, None, :] == combos[None, :, :]).mean(-1).astype(np.float64)


def _bf16(a):
    import ml_dtypes

    return np.asarray(a, dtype=np.float32).astype(ml_dtypes.bfloat16)


def build_host_constants(protos):
    """protos: dict name -> P_g [K_g, D] float32. Returns the bf16
    [128, BF_COLS] and f32 [128, F32_COLS] constant blobs shared by all
    cores."""
    import ml_dtypes

    blob32 = np.zeros((128, F32_COLS), dtype=np.float32)
    blob16 = np.zeros((128, BF_COLS), dtype=ml_dtypes.bfloat16)

    pn_pad = np.zeros((2, 128, D), dtype=np.float32)   # l2-normalized, padded
    w_pad = np.zeros((2, 128, D), dtype=np.float32)    # (A/c) @ P, padded
    ind = np.zeros((2, 128, N_GROUPS), dtype=np.float32)
    indt = np.zeros((2, N_GROUPS, 128), dtype=np.float32)

    for g, name in enumerate(GROUP_DIMS):
        P = np.asarray(protos[name], dtype=np.float32)
        K, layout, base = GROUP_PLACEMENT[name]
        assert P.shape == (K, D)
        norm = np.linalg.norm(P, axis=-1, keepdims=True).astype(np.float32)
        pn = P / (norm + np.float32(1e-6))
        A = _affinity(GROUP_DIMS[name])                 # [K, K] float64
        c = A[0].sum() + 1e-6                            # constant row sum
        W = ((A / c) @ P.astype(np.float64)).astype(np.float32)
        pn_pad[layout, base : base + K] = pn
        w_pad[layout, base : base + K] = W
        ind[layout, base : base + K, g] = 1.0
        indt[layout, g, base : base + K] = 1.0

    for c in range(N_CHUNK):
        for l in range(2):
            # pnt chunk: [128 d_local, 128 padded-row] = pn_pad[l][:, chunk].T
            blob16[:, _PNT_OFF + c * 256 + l * 128 : _PNT_OFF + c * 256 + (l + 1) * 128] = (
                _bf16(pn_pad[l][:, c * 128 : (c + 1) * 128].T)
            )
    for l in range(2):
        blob16[:, _W_OFF + l * D : _W_OFF + (l + 1) * D] = _bf16(w_pad[l])
        blob32[:, _IND_OFF + l * N_GROUPS : _IND_OFF + (l + 1) * N_GROUPS] = ind[l]
        blob32[0:N_GROUPS, _INDT_OFF + l * 128 : _INDT_OFF + (l + 1) * 128] = indt[l]
    blob32[:, _ONES_OFF : _ONES_OFF + 128] = 1.0
    return blob32, blob16


# ------------------------------------------------------------ bass program
def build_program(loop_k=None, ablate=None, repeat=1):
    """Emit the SPMD program. loop_k: if set, wrap the whole body in a
    tc.For_i repeat for delta-timing (adds a per-iteration all-engine
    barrier). repeat: python-unrolled repeats (barrier-free, matches the
    real kernel's pipelining). ablate: None | "dma_only" | "no_outdma"
    (perf diagnostics; wrong results)."""
    _install_tile_patch()
    nc = bass.Bass("TRN2", target_bir_lowering=False, debug=False,
                   num_devices=N_CORES)
    # x, pre-transposed + bf16 on host: partition p of pair r holds the
    # two supertiles' chunks contiguously (2*4*512 bf16 = 8KB descriptors)
    xt_d = nc.dram_tensor("xt", [N_SUPER // 2, 128, 2 * N_CHUNK * SUPER],
                          _BF16, kind="ExternalInput").ap()
    const32_d = nc.dram_tensor("consts32", [128, F32_COLS], _F32,
                               kind="ExternalInput").ap()
    const16_d = nc.dram_tensor("consts16", [128, BF_COLS], _BF16,
                               kind="ExternalInput").ap()
    onesr_d = nc.dram_tensor("onesr", [128, 1], _F32R,
                             kind="ExternalInput").ap()
    # out, partition-contiguous per supertile: row (s, p) holds the 4
    # j-tiles' [5, 512] slabs for batch rows s*512 + j*128 + p (20KB descs)
    out_d = nc.dram_tensor("out", [N_SUPER, 128, N_J * N_GROUPS * D],
                           _BF16, kind="ExternalOutput").ap()

    with tile.TileContext(nc) as tc:
        import contextlib

        with contextlib.ExitStack() as ctx:
            cpool = ctx.enter_context(tc.tile_pool(name="consts", bufs=1))
            xt_pool = ctx.enter_context(tc.tile_pool(name="xt", bufs=3))
            sq_pool = ctx.enter_context(tc.tile_pool(name="sq", bufs=3))
            work = ctx.enter_context(tc.tile_pool(name="work", bufs=4))
            et_pool = ctx.enter_context(tc.tile_pool(name="et", bufs=4))
            qt_pool = ctx.enter_context(tc.tile_pool(name="qt", bufs=4))
            tiny = ctx.enter_context(tc.tile_pool(name="tiny", bufs=8))
            stage = ctx.enter_context(tc.tile_pool(name="stage", bufs=3))
            ps_small = ctx.enter_context(
                tc.tile_pool(name="ps_small", bufs=PSUM_BUFS["small"],
                             space="PSUM"))
            ps_raw = ctx.enter_context(
                tc.tile_pool(name="ps_raw", bufs=PSUM_BUFS["raw"],
                             space="PSUM"))
            ps_bcast = ctx.enter_context(
                tc.tile_pool(name="ps_bcast", bufs=PSUM_BUFS["bcast"],
                             space="PSUM"))
            ps_out = ctx.enter_context(
                tc.tile_pool(name="ps_out", bufs=PSUM_BUFS["out"],
                             space="PSUM"))

            consts32 = cpool.tile([128, F32_COLS], _F32)
            nc.sync.dma_start(out=consts32[:], in_=const32_d[:])
            consts16 = cpool.tile([128, BF_COLS], _BF16)
            nc.sync.dma_start(out=consts16[:], in_=const16_d[:])

            def pnt(c, l):
                o = _PNT_OFF + c * 256 + l * 128
                return consts16[:, o : o + 128]

            def w_l(l, base, K):
                return consts16[base : base + K,
                                _W_OFF + l * D : _W_OFF + (l + 1) * D]

            def ind_l(l):
                o = _IND_OFF + l * N_GROUPS
                return consts32[:, o : o + N_GROUPS]

            def indt_l(l):
                o = _INDT_OFF + l * 128
                return consts32[0:N_GROUPS, o : o + 128]

            onesr = cpool.tile([128, 1], _F32R)
            nc.sync.dma_start(out=onesr[:], in_=onesr_d[:])
            ones_col_r = onesr[:]
            ones_row = consts32[0:1, _ONES_OFF : _ONES_OFF + 128]   # [1,128]

            groups = list(GROUP_PLACEMENT.items())

            def front(s, xt2):
                """Scale pipeline + logits + softmax for supertile s (no
                DVE qt yet — that is emitted after back(s-1) so the DVE
                FIFO does s-1's copies while s's inv chain resolves)."""
                xt = xt2[:, s % 2]          # [128, N_CHUNK, SUPER] bf16
                # squares (gpsimd, SBUF only) + ssq = ones^T . sq  (PE)
                sq = sq_pool.tile([128, N_CHUNK, SUPER], _F32R, tag="sq")
                for c in range(N_CHUNK):
                    nc.gpsimd.tensor_mul(sq[:, c], xt[:, c], xt[:, c])
                ssq = ps_small.tile([1, SUPER], _F32, tag="small")
                for c in range(N_CHUNK):
                    nc.tensor.matmul(
                        ssq[:], ones_col_r, sq[:, c],
                        start=(c == 0), stop=(c == N_CHUNK - 1))
                # s = exp(-0.5 * ln(tau^2 * ssq)) = 1/(tau*||x||)
                t1 = tiny.tile([1, SUPER], _F32, tag="tiny")
                nc.scalar.activation(t1[:], ssq[:], _LN, scale=float(TAU * TAU))
                s_t = tiny.tile([1, SUPER], _F32, tag="tiny")
                nc.scalar.activation(s_t[:], t1[:], _EXP, scale=-0.5)
                # S = broadcast of s to 128 partitions (PE, exact in f32)
                S_ps = ps_small.tile([128, SUPER], _F32, tag="small")
                nc.tensor.matmul(S_ps[:], ones_row, s_t[:], start=True,
                                 stop=True)
                S_sb = work.tile([128, SUPER], _F32, tag="S")
                nc.scalar.copy(S_sb[:], S_ps[:])

                # logits^T per layout (bf16 matmul), exp
                ets = []
                for l in range(2):
                    raw = ps_raw.tile([128, SUPER], _F32, tag="raw")
                    for c in range(N_CHUNK):
                        nc.tensor.matmul(
                            raw[:], pnt(c, l), xt[:, c],
                            start=(c == 0), stop=(c == N_CHUNK - 1))
                    rawS = work.tile([128, SUPER], _F32, tag="rawS")
                    nc.vector.tensor_mul(rawS[:], raw[:], S_sb[:])
                    et = et_pool.tile([128, SUPER], _F32, tag="et")
                    nc.scalar.activation(et[:], rawS[:], _EXP)
                    ets.append(et)

                # segment sums over both layouts -> [5, SUPER]
                sums = ps_small.tile([N_GROUPS, SUPER], _F32, tag="small")
                for l in range(2):
                    nc.tensor.matmul(sums[:], ind_l(l), ets[l][:],
                                     start=(l == 0), stop=(l == 1))
                # inv = exp(-ln(sums)) = 1/sums
                t2 = tiny.tile([N_GROUPS, SUPER], _F32, tag="tiny")
                nc.scalar.activation(t2[:], sums[:], _LN)
                inv = tiny.tile([N_GROUPS, SUPER], _F32, tag="tiny")
                nc.scalar.activation(inv[:], t2[:], _EXP, scale=-1.0)
                # invB = broadcast of inv rows to group partition ranges (PE)
                invBs = []
                for l in range(2):
                    invB = ps_bcast.tile([128, SUPER], _F32, tag="invB")
                    nc.tensor.matmul(invB[:], indt_l(l), inv[:],
                                     start=True, stop=True)
                    invBs.append(invB)
                return {"ets": ets, "invBs": invBs}

            def qt_stage(state):
                """qT per layout = eT * invB (DVE), bf16 for the PE."""
                qts = []
                for l in range(2):
                    qt = qt_pool.tile([128, SUPER], _BF16, tag="qt")
                    nc.vector.tensor_mul(qt[:], state["ets"][l][:],
                                         state["invBs"][l][:])
                    qts.append(qt)
                state["qts"] = qts

            def back(s, state):
                """Second matmul + PSUM evacuation + out DMA for supertile
                s.  Copy split (DVE 0.96GHz vs ACT 1.2GHz): DVE gets g0/g1,
                ACT g2/g3, g4 alternates by j."""
                qts = state["qts"]
                st = stage.tile([128, N_J, N_GROUPS, D], _BF16, tag="stage")
                for j in range(N_J):
                    def mm2(ps_slice, gi):
                        name, (K, l, base) = groups[gi]
                        nc.tensor.matmul(
                            ps_slice,
                            qts[l][base : base + K, j * 128 : (j + 1) * 128],
                            w_l(l, base, K),
                            start=True, stop=True)

                    pr0 = ps_out.tile([128, 2, D], _F32, tag="out")
                    mm2(pr0[:, 0, :], 0)
                    mm2(pr0[:, 1, :], 1)
                    nc.vector.tensor_copy(st[:, j, 0:2, :], pr0[:])
                    pr1 = ps_out.tile([128, 2, D], _F32, tag="out")
                    mm2(pr1[:, 0, :], 2)
                    mm2(pr1[:, 1, :], 3)
                    nc.scalar.copy(st[:, j, 2:4, :], pr1[:])
                    pr2 = ps_out.tile([128, 2, D], _F32, tag="out")
                    mm2(pr2[:, 0, :], 4)
                    if j % 2 == 0:
                        nc.vector.tensor_copy(st[:, j, 4, :], pr2[:, 0, :])
                    else:
                        nc.scalar.copy(st[:, j, 4, :], pr2[:, 0, :])
                if ablate != "no_outdma":
                    nc.sync.dma_start(
                        out=out_d[s],
                        in_=st[:].rearrange("p j g d -> p (j g d)"))

            def body():
                if ablate == "dma_only":
                    for r in range(N_SUPER // 2):
                        xt2 = xt_pool.tile([128, 2, N_CHUNK, SUPER], _BF16,
                                           tag="xt")
                        nc.sync.dma_start(
                            out=xt2[:],
                            in_=xt_d[r].rearrange("p (t c b) -> p t c b",
                                                  t=2, c=N_CHUNK))
                        for i in range(2):
                            st = stage.tile([128, N_J, N_GROUPS, D], _BF16,
                                            tag="stage")
                            nc.vector.tensor_copy(st[:, 0, 0, 0:4],
                                                  xt2[0:128, i, 0, 0:4])
                            nc.sync.dma_start(
                                out=out_d[2 * r + i],
                                in_=st[:].rearrange("p j g d -> p (j g d)"))
                    return
                # software pipeline: back(s-1) slots between front(s) and
                # qt(s) so DVE/ACT fill their waits with s-1's copies
                prev = None
                for r in range(N_SUPER // 2):
                    xt2 = xt_pool.tile([128, 2, N_CHUNK, SUPER], _BF16,
                                       tag="xt")
                    nc.sync.dma_start(
                        out=xt2[:],
                        in_=xt_d[r].rearrange("p (t c b) -> p t c b",
                                              t=2, c=N_CHUNK))
                    for i in range(2):
                        s = 2 * r + i
                        state = front(s, xt2)
                        if prev is not None:
                            back(prev[0], prev[1])
                        qt_stage(state)
                        prev = (s, state)
                back(prev[0], prev[1])

            if loop_k is None:
                for _ in range(repeat):
                    body()
            else:
                with tc.For_i(0, loop_k, 1):
                    body()

    return nc


# ------------------------------------------------------------- entry point
def make_core_inputs(inputs):
    """Full inputs dict -> list of 8 per-core input maps."""
    import ml_dtypes

    x = np.asarray(inputs["x"], dtype=np.float32)
    assert x.shape == (B, D)
    protos = {k: inputs[f"P_{k}"] for k in GROUP_DIMS}
    blob32, blob16 = build_host_constants(protos)
    maps = []
    for i in range(N_CORES):
        shard = x[i * B_CORE : (i + 1) * B_CORE]
        # [B_CORE, D] -> xT [D, B_CORE] -> [c, p, r, t*b] -> [r, p, (t c b)]
        xt = shard.T.reshape(N_CHUNK, 128, N_SUPER // 2, 2, SUPER)
        xt = np.ascontiguousarray(xt.transpose(2, 1, 3, 0, 4)).astype(
            ml_dtypes.bfloat16)
        maps.append({
            "xt": xt.reshape(N_SUPER // 2, 128, 2 * N_CHUNK * SUPER),
            "consts32": blob32,
            "consts16": blob16,
            "onesr": np.ones((128, 1), dtype=np.float32),
        })
    return maps


def assemble_output(res_list):
    """Per-core 'out' tensors -> full [B, N_GROUPS, D] float32."""
    outs = []
    for i in range(N_CORES):
        a = np.asarray(res_list[i]["out"]).reshape(
            N_SUPER, 128, N_J, N_GROUPS, D)
        a = a.transpose(0, 2, 1, 3, 4).reshape(B_CORE, N_GROUPS, D)
        outs.append(a.astype(np.float32))
    return np.concatenate(outs, axis=0)


_CACHE = {}


def kernel(x, P_gender, P_hair, P_top, P_pants, P_shoes):
    from concourse.bass_utils import run_bass_kernel_spmd

    inputs = dict(x=x, P_gender=P_gender, P_hair=P_hair, P_top=P_top,
                  P_pants=P_pants, P_shoes=P_shoes)
    if "nc" not in _CACHE:
        _CACHE["nc"] = build_program()
    in_maps = make_core_inputs(inputs)
    res = run_bass_kernel_spmd(_CACHE["nc"], in_maps, list(range(N_CORES)))
    return assemble_output(res.results)


# revision 21
# speedup vs baseline: 4.1306x; 1.0558x over previous
"""Trainium2 Bass kernel for nn_CerberusSemanticIDBranch (vq_codebook).

Reference semantics (per group g with prototypes P_g [K_g, D]):
    xn = x / (||x|| + 1e-6)
    logits = (xn @ l2(P_g).T) / tau
    q = softmax(logits)
    q_aff = q @ A_g;  q_aff /= (q_aff.sum(-1) + 1e-6)
    out_g = q_aff @ P_g
stacked over 5 groups -> [B, 5, D].

Key folds (host side):
  * A_g has constant row-sums c_g, so the affinity normalization is a
    constant scale: out_g = q @ W_g with W_g = (A_g/(c_g+1e-6)) @ P_g.
  * l2(P) is precomputed; 1/tau and 1/||x|| fold into one per-row scale
    s_b = 1/(tau*||x_b||), applied to the logits.
  * softmax denominator via exp/ln only (single ACT table set):
    s = exp(-0.5*ln(tau^2*ssq)), inv = exp(-ln(segsum)).

Memory regime: the kernel is HBM-bound (out is 41.9MB/core in fp32), so
x, the prototype constants, and the output all travel as bf16 (the
harness gate is max|err|/absmax < 2e-2; bf16 costs ~3e-3).  The output
is written in a partition-contiguous layout ([N_SUPER, 128, 4*5*512],
one 2.6MB DMA per supertile with 20KB/partition descriptors) and
unshuffled + upcast on the host.

Device layout: x is pre-transposed on the host so D sits on SBUF
partitions; batch stays on the free dim through the logits/softmax,
so per-group segment sums are partition-axis sums done as PE matmuls
with 0/1 indicator matrices.  The 5 groups are padded into two
128-partition layouts (A: top@0, pants@64 / B: gender@0, hair@32,
shoes@64) so every matmul operand has a 0/32/64-aligned base partition.

Data parallel over 8 NeuronCores: core i handles rows [i*4096, (i+1)*4096).
"""

import itertools
import sys

import numpy as np

sys.path.insert(0, "/opt/trn_rl_repo")

import concourse.bass as bass  # noqa: E402
import concourse.tile as tile  # noqa: E402
from concourse import mybir  # noqa: E402
from concourse.vector_clock import ScopedClock  # noqa: E402

# ---------------------------------------------------------------- problem
GROUP_DIMS = {
    "gender": [2],
    "hair": [5, 3],
    "top": [8, 5],
    "pants": [8, 5],
    "shoes": [6, 4],
}
TAU = 0.07
B, D = 32768, 512
N_CORES = 8
B_CORE = B // N_CORES          # 4096
SUPER = 512                    # batch rows per supertile
N_SUPER = B_CORE // SUPER      # 8
N_CHUNK = D // 128             # 4
N_GROUPS = 5
N_J = SUPER // 128             # 4 row-tiles per supertile

# group -> (K, layout, base partition); layout A=0, B=1
GROUP_PLACEMENT = {
    "gender": (2, 1, 0),
    "hair": (15, 1, 32),
    "top": (40, 0, 0),
    "pants": (40, 0, 64),
    "shoes": (24, 1, 64),
}

# bf16 const blob column offsets ([128, BF_COLS] bfloat16)
_PNT_OFF = 0                       # 4 chunks x (A 128 | B 128) = 1024
_W_OFF = 1024                      # 2 layouts x 512
BF_COLS = 2048
# f32 const blob column offsets ([128, F32_COLS] float32)
_IND_OFF = 0                       # 2 layouts x 5
_INDT_OFF = 10                     # 2 layouts x 128 (rows 0:5)
_ONES_OFF = 266                    # [128, 128] of ones
F32_COLS = 394

PSUM_BUFS = {"small": 2, "raw": 1, "bcast": 1, "out": 2}  # out slots are 2 banks

_F32 = mybir.dt.float32
_F32R = mybir.dt.float32r
_BF16 = mybir.dt.bfloat16
_EXP = mybir.ActivationFunctionType.Exp
_LN = mybir.ActivationFunctionType.Ln


# ------------------------------------------------------------- tile patch
_NOP_ID = [0]


def _spread_all_waits(nc, max_waits=1):
    """This walrus build rejects instructions carrying more than one sync
    wait (setupSyncWait: "Too many sync wait commands").  Rewrite every
    block so extra waits ride on dedicated same-engine NOPs placed just
    before the instruction (engine queues are FIFO, so semantics hold)."""
    for fn in nc.m.functions:
        for blk in fn.blocks:
            insts = list(blk.instructions)
            out = []
            changed = False
            for inst in insts:
                si = inst.sync_info
                waits = list(si.on_wait) if si is not None and si.on_wait else []
                if len(waits) > max_waits:
                    changed = True
                    for w in waits[:-max_waits]:
                        _NOP_ID[0] += 1
                        out.append(
                            mybir.InstNoOp(
                                name=f"waitnop-{_NOP_ID[0]}",
                                engine=inst.engine,
                                bass_nofuse=True,
                                sync_info=mybir.SyncInfo(
                                    on_wait=[w], on_update=[]),
                            ))
                    si.on_wait = waits[-max_waits:]
                out.append(inst)
            if changed:
                blk.instructions = out


def _patched_drain_and_barrier(self, tick_clock, wait_clock):
    probe = self.nc.sync.nop(nofuse=True)
    wait_clock.add_sem_waits(probe.ins, ScopedClock({None: tick_clock.global_clock}))
    drain_inst = self.nc.sync.drain()
    del drain_inst
    self.nc.all_engine_barrier()
    assert self.sems is not None
    popped = self.nc._tile_sem_poison_stack.pop()
    assert popped is self._sem_poison
    self.nc.clear_and_free_semaphores(list(self.sems.allocated().values()))
    self.nc.all_engine_barrier()
    _spread_all_waits(self.nc)


_patched = False


def _install_tile_patch():
    global _patched
    if not _patched:
        tile.TileContext._drain_and_barrier = _patched_drain_and_barrier
        _patched = True


# --------------------------------------------------------- host constants
def _affinity(dims):
    combos = np.array(
        list(itertools.product(*[range(d) for d in dims])), dtype=np.int32
    )
    return (combos[:, None, :] == combos[None, :, :]).mean(-1).astype(np.float64)


def _bf16(a):
    import ml_dtypes

    return np.asarray(a, dtype=np.float32).astype(ml_dtypes.bfloat16)


def build_host_constants(protos):
    """protos: dict name -> P_g [K_g, D] float32. Returns the bf16
    [128, BF_COLS] and f32 [128, F32_COLS] constant blobs shared by all
    cores."""
    import ml_dtypes

    blob32 = np.zeros((128, F32_COLS), dtype=np.float32)
    blob16 = np.zeros((128, BF_COLS), dtype=ml_dtypes.bfloat16)

    pn_pad = np.zeros((2, 128, D), dtype=np.float32)   # l2-normalized, padded
    w_pad = np.zeros((2, 128, D), dtype=np.float32)    # (A/c) @ P, padded
    ind = np.zeros((2, 128, N_GROUPS), dtype=np.float32)
    indt = np.zeros((2, N_GROUPS, 128), dtype=np.float32)

    for g, name in enumerate(GROUP_DIMS):
        P = np.asarray(protos[name], dtype=np.float32)
        K, layout, base = GROUP_PLACEMENT[name]
        assert P.shape == (K, D)
        norm = np.linalg.norm(P, axis=-1, keepdims=True).astype(np.float32)
        pn = P / (norm + np.float32(1e-6))
        A = _affinity(GROUP_DIMS[name])                 # [K, K] float64
        c = A[0].sum() + 1e-6                            # constant row sum
        W = ((A / c) @ P.astype(np.float64)).astype(np.float32)
        pn_pad[layout, base : base + K] = pn
        w_pad[layout, base : base + K] = W
        ind[layout, base : base + K, g] = 1.0
        indt[layout, g, base : base + K] = 1.0

    for c in range(N_CHUNK):
        for l in range(2):
            # pnt chunk: [128 d_local, 128 padded-row] = pn_pad[l][:, chunk].T
            blob16[:, _PNT_OFF + c * 256 + l * 128 : _PNT_OFF + c * 256 + (l + 1) * 128] = (
                _bf16(pn_pad[l][:, c * 128 : (c + 1) * 128].T)
            )
    for l in range(2):
        blob16[:, _W_OFF + l * D : _W_OFF + (l + 1) * D] = _bf16(w_pad[l])
        blob32[:, _IND_OFF + l * N_GROUPS : _IND_OFF + (l + 1) * N_GROUPS] = ind[l]
        blob32[0:N_GROUPS, _INDT_OFF + l * 128 : _INDT_OFF + (l + 1) * 128] = indt[l]
    blob32[:, _ONES_OFF : _ONES_OFF + 128] = 1.0
    return blob32, blob16


# ------------------------------------------------------------ bass program
def build_program(loop_k=None, ablate=None, repeat=1, interleave=False):
    """Emit the SPMD program. loop_k: if set, wrap the whole body in a
    tc.For_i repeat for delta-timing (adds a per-iteration all-engine
    barrier). repeat: python-unrolled repeats (barrier-free, matches the
    real kernel's pipelining). ablate: None | "dma_only" | "no_outdma"
    (perf diagnostics; wrong results)."""
    _install_tile_patch()
    nc = bass.Bass("TRN2", target_bir_lowering=False, debug=False,
                   num_devices=N_CORES)
    # x, pre-transposed + bf16 on host: partition p of pair r holds the
    # two supertiles' chunks contiguously (2*4*512 bf16 = 8KB descriptors)
    xt_d = nc.dram_tensor("xt", [N_SUPER // 2, 128, 2 * N_CHUNK * SUPER],
                          _BF16, kind="ExternalInput").ap()
    constr_d = nc.dram_tensor("constsr", [128, F32_COLS], _F32R,
                              kind="ExternalInput").ap()
    const16_d = nc.dram_tensor("consts16", [128, BF_COLS], _BF16,
                               kind="ExternalInput").ap()
    onesr_d = nc.dram_tensor("onesr", [128, 1], _F32R,
                             kind="ExternalInput").ap()
    # out, partition-contiguous per supertile: row (s, p) holds the 4
    # j-tiles' [5, 512] slabs for batch rows s*512 + j*128 + p (20KB descs)
    out_d = nc.dram_tensor("out", [N_SUPER, 128, N_J * N_GROUPS * D],
                           _BF16, kind="ExternalOutput").ap()

    with tile.TileContext(nc) as tc:
        import contextlib

        with contextlib.ExitStack() as ctx:
            cpool = ctx.enter_context(tc.tile_pool(name="consts", bufs=1))
            xt_pool = ctx.enter_context(tc.tile_pool(name="xt", bufs=3))
            sq_pool = ctx.enter_context(tc.tile_pool(name="sq", bufs=3))
            work = ctx.enter_context(tc.tile_pool(name="work", bufs=4))
            et_pool = ctx.enter_context(tc.tile_pool(name="et", bufs=4))
            qt_pool = ctx.enter_context(tc.tile_pool(name="qt", bufs=4))
            tiny = ctx.enter_context(tc.tile_pool(name="tiny", bufs=8))
            stage = ctx.enter_context(tc.tile_pool(name="stage", bufs=3))
            ps_small = ctx.enter_context(
                tc.tile_pool(name="ps_small", bufs=PSUM_BUFS["small"],
                             space="PSUM"))
            ps_raw = ctx.enter_context(
                tc.tile_pool(name="ps_raw", bufs=PSUM_BUFS["raw"],
                             space="PSUM"))
            ps_bcast = ctx.enter_context(
                tc.tile_pool(name="ps_bcast", bufs=PSUM_BUFS["bcast"],
                             space="PSUM"))
            ps_out = ctx.enter_context(
                tc.tile_pool(name="ps_out", bufs=PSUM_BUFS["out"],
                             space="PSUM"))

            constsr = cpool.tile([128, F32_COLS], _F32R)
            nc.sync.dma_start(out=constsr[:], in_=constr_d[:])
            consts16 = cpool.tile([128, BF_COLS], _BF16)
            nc.sync.dma_start(out=consts16[:], in_=const16_d[:])

            def pnt(c, l):
                o = _PNT_OFF + c * 256 + l * 128
                return consts16[:, o : o + 128]

            def w_l(l, base, K):
                return consts16[base : base + K,
                                _W_OFF + l * D : _W_OFF + (l + 1) * D]

            def ind_l(l):
                o = _IND_OFF + l * N_GROUPS
                return constsr[:, o : o + N_GROUPS]

            def indt_l(l):
                o = _INDT_OFF + l * 128
                return constsr[0:N_GROUPS, o : o + 128]

            onesr = cpool.tile([128, 1], _F32R)
            nc.sync.dma_start(out=onesr[:], in_=onesr_d[:])
            ones_col_r = onesr[:]
            ones_row = constsr[0:1, _ONES_OFF : _ONES_OFF + 128]   # [1,128]

            groups = list(GROUP_PLACEMENT.items())

            def front1(s, xt2):
                """Front part 1: squares + ssq + ln/exp of the row-norm
                scale.  Parts 2/3 and the previous supertile's back-half
                j-tiles are interleaved by the caller so each engine's
                FIFO alternates chain ops with ready copy work."""
                xt = xt2[:, s % 2]          # [128, N_CHUNK, SUPER] bf16
                # squares (gpsimd, SBUF only) + ssq = ones^T . sq  (PE)
                sq = sq_pool.tile([128, N_CHUNK, SUPER], _F32R, tag="sq")
                for c in range(N_CHUNK):
                    nc.gpsimd.tensor_mul(sq[:, c], xt[:, c], xt[:, c])
                ssq_t = ps_small.tile([1, SUPER], _F32, tag="small")
                ssq = ssq_t[:]
                for c in range(N_CHUNK):
                    nc.tensor.matmul(
                        ssq, ones_col_r, sq[:, c],
                        start=(c == 0), stop=(c == N_CHUNK - 1))
                # s = exp(-0.5 * ln(tau^2 * ssq)) = 1/(tau*||x||)
                t1 = tiny.tile([1, SUPER], _F32, tag="tiny")
                nc.scalar.activation(t1[:], ssq, _LN, scale=float(TAU * TAU))
                s_t = tiny.tile([1, SUPER], _F32R, tag="tiny")
                nc.scalar.activation(s_t[:], t1[:], _EXP, scale=-0.5)
                return {"xt": xt, "s_t": s_t}

            def front2(state):
                xt, s_t = state["xt"], state["s_t"]
                # S = broadcast of s to 128 partitions (PE, exact in f32)
                S_ps = ps_small.tile([128, SUPER], _F32, tag="small")
                nc.tensor.matmul(S_ps[:], ones_row, s_t[:], start=True,
                                 stop=True)
                S_sb = work.tile([128, SUPER], _F32, tag="S")
                nc.scalar.copy(S_sb[:], S_ps[:])

                # logits^T per layout (bf16 matmul), exp
                ets = []
                for l in range(2):
                    raw = ps_raw.tile([128, SUPER], _F32, tag="raw")
                    for c in range(N_CHUNK):
                        nc.tensor.matmul(
                            raw[:], pnt(c, l), xt[:, c],
                            start=(c == 0), stop=(c == N_CHUNK - 1))
                    rawS = work.tile([128, SUPER], _F32, tag="rawS")
                    nc.vector.tensor_mul(rawS[:], raw[:], S_sb[:])
                    et = et_pool.tile([128, SUPER], _F32R, tag="et")
                    nc.scalar.activation(et[:], rawS[:], _EXP)
                    ets.append(et[:])
                del et
                state["ets"] = ets

            def front3(state):
                ets = state["ets"]
                # segment sums over both layouts -> [5, SUPER]
                sums_t = ps_small.tile([N_GROUPS, SUPER], _F32, tag="small")
                sums = sums_t[:]
                for l in range(2):
                    nc.tensor.matmul(sums, ind_l(l), ets[l],
                                     start=(l == 0), stop=(l == 1))
                # inv = exp(-ln(sums)) = 1/sums
                t2 = tiny.tile([N_GROUPS, SUPER], _F32, tag="tiny")
                nc.scalar.activation(t2[:], sums, _LN)
                inv = tiny.tile([N_GROUPS, SUPER], _F32R, tag="tiny")
                nc.scalar.activation(inv[:], t2[:], _EXP, scale=-1.0)
                # invB = broadcast of inv rows to group partition ranges (PE)
                invBs = []
                for l in range(2):
                    invB = ps_bcast.tile([128, SUPER], _F32, tag="invB")
                    nc.tensor.matmul(invB[:], indt_l(l), inv[:],
                                     start=True, stop=True)
                    invBs.append(invB)
                state["invBs"] = invBs

            def qt_stage(state):
                """qT per layout = eT * invB (DVE), bf16 for the PE."""
                qts = []
                for l in range(2):
                    qt = qt_pool.tile([128, SUPER], _BF16, tag="qt")
                    nc.vector.tensor_mul(qt[:], state["ets"][l],
                                         state["invBs"][l][:])
                    qts.append(qt)
                state["qts"] = qts

            def back_js(s, state, js):
                """Second matmul + PSUM evacuation for j-tiles `js` of
                supertile s.  Copy split (DVE 0.96GHz vs ACT 1.2GHz): DVE
                gets g0/g1, ACT g2/g3, g4 alternates by j."""
                qts = state["qts"]
                if "st" not in state:
                    st_tile = stage.tile([128, N_J, N_GROUPS, D], _BF16,
                                         tag="stage")
                    state["st"] = st_tile
                st = state["st"]
                for j in js:
                    def mm2(ps_slice, gi):
                        name, (K, l, base) = groups[gi]
                        nc.tensor.matmul(
                            ps_slice,
                            qts[l][base : base + K, j * 128 : (j + 1) * 128],
                            w_l(l, base, K),
                            start=True, stop=True)

                    pr0 = ps_out.tile([128, 2, D], _F32, tag="out")
                    mm2(pr0[:, 0, :], 0)
                    mm2(pr0[:, 1, :], 1)
                    nc.vector.tensor_copy(st[:, j, 0:2, :], pr0[:])
                    pr1 = ps_out.tile([128, 2, D], _F32, tag="out")
                    mm2(pr1[:, 0, :], 2)
                    mm2(pr1[:, 1, :], 3)
                    nc.scalar.copy(st[:, j, 2:4, :], pr1[:])
                    pr2 = ps_out.tile([128, 2, D], _F32, tag="out")
                    mm2(pr2[:, 0, :], 4)
                    if j % 2 == 0:
                        nc.vector.tensor_copy(st[:, j, 4, :], pr2[:, 0, :])
                    else:
                        nc.scalar.copy(st[:, j, 4, :], pr2[:, 0, :])
                if js[-1] == N_J - 1 and ablate != "no_outdma":
                    nc.sync.dma_start(
                        out=out_d[s],
                        in_=st[:].rearrange("p j g d -> p (j g d)"))

            def body():
                if ablate == "dma_only":
                    for r in range(N_SUPER // 2):
                        xt2 = xt_pool.tile([128, 2, N_CHUNK, SUPER], _BF16,
                                           tag="xt")
                        nc.sync.dma_start(
                            out=xt2[:],
                            in_=xt_d[r].rearrange("p (t c b) -> p t c b",
                                                  t=2, c=N_CHUNK))
                        for i in range(2):
                            st = stage.tile([128, N_J, N_GROUPS, D], _BF16,
                                            tag="stage")
                            nc.vector.tensor_copy(st[:, 0, 0, 0:4],
                                                  xt2[0:128, i, 0, 0:4])
                            nc.sync.dma_start(
                                out=out_d[2 * r + i],
                                in_=st[:].rearrange("p j g d -> p (j g d)"))
                    return
                # software pipeline with fine interleaving: supertile
                # s-1's back-half j-tiles slot between s's front stages so
                # DVE/ACT always have ready copy work while s's serial
                # scale/softmax chain resolves
                prev = None
                for r in range(N_SUPER // 2):
                    xt2 = xt_pool.tile([128, 2, N_CHUNK, SUPER], _BF16,
                                       tag="xt")
                    nc.sync.dma_start(
                        out=xt2[:],
                        in_=xt_d[r].rearrange("p (t c b) -> p t c b",
                                              t=2, c=N_CHUNK))
                    for i in range(2):
                        s = 2 * r + i
                        if interleave:
                            state = front1(s, xt2)
                            if prev is not None:
                                back_js(prev[0], prev[1], [0, 1])
                            front2(state)
                            if prev is not None:
                                back_js(prev[0], prev[1], [2, 3])
                            front3(state)
                            qt_stage(state)
                        else:
                            state = front1(s, xt2)
                            front2(state)
                            front3(state)
                            if prev is not None:
                                back_js(prev[0], prev[1], [0, 1, 2, 3])
                            qt_stage(state)
                        prev = (s, state)
                back_js(prev[0], prev[1], [0, 1])
                back_js(prev[0], prev[1], [2, 3])
                del prev

            if loop_k is None:
                for _ in range(repeat):
                    body()
            else:
                with tc.For_i(0, loop_k, 1):
                    body()

    return nc


# ------------------------------------------------------------- entry point
def make_core_inputs(inputs):
    """Full inputs dict -> list of 8 per-core input maps."""
    import ml_dtypes

    x = np.asarray(inputs["x"], dtype=np.float32)
    assert x.shape == (B, D)
    protos = {k: inputs[f"P_{k}"] for k in GROUP_DIMS}
    blob32, blob16 = build_host_constants(protos)
    maps = []
    for i in range(N_CORES):
        shard = x[i * B_CORE : (i + 1) * B_CORE]
        # [B_CORE, D] -> xT [D, B_CORE] -> [c, p, r, t*b] -> [r, p, (t c b)]
        xt = shard.T.reshape(N_CHUNK, 128, N_SUPER // 2, 2, SUPER)
        xt = np.ascontiguousarray(xt.transpose(2, 1, 3, 0, 4)).astype(
            ml_dtypes.bfloat16)
        maps.append({
            "xt": xt.reshape(N_SUPER // 2, 128, 2 * N_CHUNK * SUPER),
            "constsr": blob32,
            "consts16": blob16,
            "onesr": np.ones((128, 1), dtype=np.float32),
        })
    return maps


def assemble_output(res_list):
    """Per-core 'out' tensors -> full [B, N_GROUPS, D] float32."""
    outs = []
    for i in range(N_CORES):
        a = np.asarray(res_list[i]["out"]).reshape(
            N_SUPER, 128, N_J, N_GROUPS, D)
        a = a.transpose(0, 2, 1, 3, 4).reshape(B_CORE, N_GROUPS, D)
        outs.append(a.astype(np.float32))
    return np.concatenate(outs, axis=0)


_CACHE = {}


def kernel(x, P_gender, P_hair, P_top, P_pants, P_shoes):
    from concourse.bass_utils import run_bass_kernel_spmd

    inputs = dict(x=x, P_gender=P_gender, P_hair=P_hair, P_top=P_top,
                  P_pants=P_pants, P_shoes=P_shoes)
    if "nc" not in _CACHE:
        _CACHE["nc"] = build_program()
    in_maps = make_core_inputs(inputs)
    res = run_bass_kernel_spmd(_CACHE["nc"], in_maps, list(range(N_CORES)))
    return assemble_output(res.results)
